# revision 12
# baseline (speedup 1.0000x reference)
"""GPTQ-style grouped-dequant linear on 8 Trainium2 cores.

out[m,n] = sum_k A[m,k] * (q[n,k] - zeros[n,k//128]) * scales[n,k//128] + bias[n]
M=2048, K=4096, N=4096, group=128.

Sharding: column-parallel — qweight/scales/zeros/bias split along N (512/core),
A replicated. Host does layout permutes + dtype casts only: A pre-cast to bf16
(same rounding the device matmul path applies anyway), q repacked to uint8,
scales/zeros pre-broadcast along the 128 k-partitions (pure replication) so the
device never spends PE time on rank-1 broadcast matmuls.

Per core: dequant is two DVE tensor_tensor ops per k-group producing bf16 W^T
tiles in [k,n] layout; the only PE work is the 512 productive 128x128x512
matmuls (16 m-tiles x 32 k-groups) accumulated in PSUM, issued so the PE goes
dense immediately (8 staggered lead tiles + catch-up bursts; a short dummy-MM
spin releases the HAM clock gate during the DMA-bound first microseconds).
DMA is spread over four issue engines / HW queues: W-side + lead A tiles on
sync (delivery order == consumption order), q on gpsimd, phase-2 A tiles on
scalar, output stores on vector (so they never block A-tile transfers).
Bias is folded into the PSUM->SBUF eviction (DVE add against a
host-replicated bias tile); output is written bf16 and upcast on host.
"""

import numpy as np
import ml_dtypes

import concourse.bass as bass
import concourse.mybir as mybir
import concourse.tile as tile
from concourse import bacc
from concourse.bass_utils import run_bass_kernel_spmd

P = 128
M, K, N = 2048, 4096, 4096
NCORES = 8
NS = N // NCORES          # 512 out-features per core
G = K // P                # 32 groups (group_size == P == 128)
MT = M // P               # 16 output row tiles

NLEAD = 8                 # lead m-tiles resident in PSUM during warmup
JOIN_AT = {0: 0, 1: 2, 2: 5, 3: 7, 4: 10, 5: 12, 6: 15, 7: 17}
SZPLAN = [1, 1, 2, 4, 4, 4, 4, 4, 4, 4]   # groups per scales/zeros DMA chunk
QPLAN = [1, 1, 2, 4, 8, 8, 8]             # groups per qweight DMA chunk
NDUMMY = 16               # warmup matmuls to release the HAM clock gate

_cached = None


def _build():
    nc = bacc.Bacc("TRN2", target_bir_lowering=False, debug=False,
                   num_devices=NCORES)
    at = nc.dram_tensor("AT4", [MT, P, G, P], mybir.dt.bfloat16,
                        kind="ExternalInput")
    qt = nc.dram_tensor("q4", [P, G, NS], mybir.dt.uint8,
                        kind="ExternalInput")
    st = nc.dram_tensor("srep", [P, G, NS], mybir.dt.bfloat16,
                        kind="ExternalInput")
    zt = nc.dram_tensor("zrep", [P, G, NS], mybir.dt.bfloat16,
                        kind="ExternalInput")
    bi = nc.dram_tensor("brep", [P, NS], mybir.dt.float32,
                        kind="ExternalInput")
    out = nc.dram_tensor("out", [M, NS], mybir.dt.bfloat16,
                         kind="ExternalOutput")

    bf16, f32 = mybir.dt.bfloat16, mybir.dt.float32

    with tile.TileContext(nc) as tc:
        with (
            tc.tile_pool(name="const", bufs=1) as const,
            tc.tile_pool(name="qpool", bufs=1) as qpool,
            tc.tile_pool(name="tmp", bufs=3) as tmpp,
            tc.tile_pool(name="wt", bufs=1) as wtp,
            tc.tile_pool(name="apool", bufs=NLEAD + 1) as apool,
            tc.tile_pool(name="mpsum", bufs=8, space="PSUM") as mpsum,
            tc.tile_pool(name="opool", bufs=3) as opool,
        ):
            srep = const.tile([P, G, NS], bf16, tag="srep")
            zrep = const.tile([P, G, NS], bf16, tag="zrep")
            q8s = qpool.tile([P, G, NS], mybir.dt.uint8, tag="q8s")
            bias_r = const.tile([P, NS], f32, tag="bias_r")
            scratch = const.tile([P, NS], bf16, tag="scratch")
            nc.vector.memset(scratch, 0.0)
            sr, zr, qr = st.ap(), zt.ap(), qt.ap()
            atr = at.ap()  # [MT, P, G, P], per-partition contiguous

            # qweight chunks ride the gpsimd (SWDGE) queue, fully parallel to
            # the sync stream below
            qg = 0
            for n in QPLAN:
                nc.gpsimd.dma_start(out=q8s[:, qg:qg + n, :],
                                    in_=qr[:, qg:qg + n, :])
                qg += n

            def load_ab(mt, eng, split=1):
                ab = apool.tile([P, G, P], bf16)
                for h in range(split):
                    g0, g1 = h * (G // split), (h + 1) * (G // split)
                    eng.dma_start(out=ab[:, g0:g1, :], in_=atr[mt, :, g0:g1, :])
                return ab

            # sync queue: delivery order == consumption order — ab0 (split for
            # early first chunks), then scales/zeros chunks for early groups
            # interleaved with the remaining lead A tiles
            lead_ab = [None] * NLEAD
            lead_ab[0] = load_ab(0, nc.sync, split=4)
            szg = 0
            for c, n in enumerate(SZPLAN):
                g0, g1 = szg, szg + n
                nc.sync.dma_start(out=srep[:, g0:g1, :], in_=sr[:, g0:g1, :])
                nc.sync.dma_start(out=zrep[:, g0:g1, :], in_=zr[:, g0:g1, :])
                szg += n
                if c + 1 < NLEAD:
                    lead_ab[c + 1] = load_ab(c + 1, nc.sync)
            nc.sync.dma_start(out=bias_r[:], in_=bi.ap()[:])

            def new_ps():
                ps = mpsum.tile([P, NS], f32)
                return ps

            # warmup spin: release the HAM clock gate while DMA streams in
            dummy_ps = new_ps()
            for i in range(NDUMMY):
                nc.tensor.matmul(dummy_ps[:], scratch[:, :P], scratch[:],
                                 start=(i == 0), stop=(i == NDUMMY - 1))

            lead_ps = [new_ps() for _ in range(NLEAD)]

            def finish(mt, ps):
                ob = opool.tile([P, NS], bf16)
                nc.vector.tensor_tensor(ob[:], ps[:], bias_r[:],
                                        mybir.AluOpType.add)
                nc.gpsimd.dma_start(out=out.ap()[mt * P:(mt + 1) * P, :],
                                    in_=ob[:])

            # Phase 1: dequant each k-group on DVE, immediately consumed by
            # the lead tiles' PSUM accumulation chains (catch-up bursts as
            # each lead joins keep the PE dense).
            wts = []
            for g in range(G):
                tmp = tmpp.tile([P, NS], bf16)
                nc.vector.tensor_tensor(tmp[:], q8s[:, g, :], zrep[:, g, :],
                                        mybir.AluOpType.subtract)
                wt = wtp.tile([P, NS], bf16, tag=f"wt{g}")
                nc.vector.tensor_tensor(wt[:], tmp[:], srep[:, g, :],
                                        mybir.AluOpType.mult)
                wts.append(wt)
                for mt in range(NLEAD):
                    if JOIN_AT[mt] == g:
                        for gc in range(g + 1):  # catch-up burst
                            nc.tensor.matmul(lead_ps[mt][:],
                                             lead_ab[mt][:, gc, :], wts[gc][:],
                                             start=(gc == 0),
                                             stop=(gc == G - 1))
                    elif JOIN_AT[mt] < g:
                        nc.tensor.matmul(lead_ps[mt][:], lead_ab[mt][:, g, :],
                                         wt[:], start=False,
                                         stop=(g == G - 1))

            # prefetch the first phase-2 A tiles on the scalar queue before
            # the lead evictions are issued (no sem-waits ahead of them)
            pre = {}
            for mt in range(NLEAD, min(NLEAD + 3, MT)):
                pre[mt] = load_ab(mt, nc.scalar)

            for mt in range(NLEAD):
                finish(mt, lead_ps[mt])

            # Phase 2: remaining output tiles, dense back-to-back matmuls
            for mt in range(NLEAD, MT):
                ab = pre.pop(mt)
                nxt = mt + 3
                if nxt < MT:
                    pre[nxt] = load_ab(nxt, nc.scalar)
                ps = new_ps()
                for g in range(G):
                    nc.tensor.matmul(ps[:], ab[:, g, :], wts[g][:],
                                     start=(g == 0), stop=(g == G - 1))
                finish(mt, ps)

    nc.compile()
    return nc


def _prep_inputs(A, qweight, scales, zeros, bias):
    # AT4[mt, p, g, j] = A[mt*128+j, g*128+p]  (layout permute + bf16 cast)
    at4 = np.ascontiguousarray(
        A.reshape(MT, P, G, P).transpose(0, 3, 2, 1).astype(ml_dtypes.bfloat16))
    in_maps = []
    for c in range(NCORES):
        r = slice(c * NS, (c + 1) * NS)
        # q4[p, g, n] = q[n, g*128+p]
        q4 = np.ascontiguousarray(
            qweight[r].astype(np.uint8).T.reshape(G, P, NS).transpose(1, 0, 2))
        # scales/zeros pre-broadcast across the 128 k-partitions (replication)
        srep = np.ascontiguousarray(np.broadcast_to(
            scales[r].T.astype(ml_dtypes.bfloat16)[None, :, :], (P, G, NS)))
        zrep = np.ascontiguousarray(np.broadcast_to(
            zeros[r].T.astype(ml_dtypes.bfloat16)[None, :, :], (P, G, NS)))
        brep = np.ascontiguousarray(np.broadcast_to(
            bias[r].astype(np.float32)[None, :], (P, NS)))
        in_maps.append({
            "AT4": at4,
            "q4": q4,
            "srep": srep,
            "zrep": zrep,
            "brep": brep,
        })
    return in_maps


def run(inputs, **spmd_kwargs):
    global _cached
    if _cached is None:
        _cached = _build()
    in_maps = _prep_inputs(**inputs)
    res = run_bass_kernel_spmd(_cached, in_maps, list(range(NCORES)),
                               **spmd_kwargs)
    outp = np.concatenate(
        [res.results[c]["out"].astype(np.float32) for c in range(NCORES)],
        axis=1)
    return outp, res


def kernel(**inputs):
    return run(inputs)[0]
